# revision 1
# baseline (speedup 1.0000x reference)
"""Trainium2 Bass kernel: y = LP(square(BP(x))) cascaded-biquad IIR filtering.

x: [16, 16384, 64] fp32; bp_sos/lp_sos: [2, 6] second-order sections.
Reference applies, per (batch, channel) sequence along time:
  w = sosfilt(w, bp_sos); w = sosfilt(w*w, lp_sos)
with zero initial conditions (Direct Form I biquads).

Strategy (exact block state-space reformulation, no sequential scan on device):
  Each 2-biquad cascade == one order-4 IIR (b, a of length 5). For chunk
  length L=128 over T=16384 (N=128 chunks), with 8-dim chunk-boundary state
  sigma_c = (u[-4..-1], y[-4..-1]):
     Y_zs   = T_h @ X            (lower-tri Toeplitz of impulse response; PE)
     d_c    = (X rows L-4..L-1, Y_zs rows L-4..L-1)   (free: just copies)
     sigma_c = sum_{j<c} M8^{c-1-j} d_j               (block-Toeplitz matmul; PE)
     Y      = Y_zs + Phi @ sigma                      (zero-input correction; PE)
  All constants (T_h, Phi, M8 powers) are computed on host in float64 from the
  runtime sos inputs. The reformulation is algebraically exact.

Sharding: data-parallel over the 1024 = 16*64 independent sequences;
128 sequences per core = SBUF partition dim. Device layout per core:
  Xm[l, c*128 + s] = x_seq[s, c*128 + l]   ([128, 16384] fp32, contiguous).
Host does the (cheap) layout transposes; device does all filtering math.
"""

import numpy as np

# ---------------------------------------------------------------- constants
B, T, C = 16, 16384, 64
NCORES = 8
L = 128           # chunk length
N = T // L        # 128 chunks per sequence
S = 128           # sequences per core
CB = 16           # chunks per state-block (CB*8 = 128 partition rows)
NB = N // CB      # 8 state blocks
TILE = 512        # matmul moving free-dim (one PSUM bank of fp32)
NT = (N * S) // TILE  # 32 column tiles


def _combine_sos(sos):
    """[2,6] sos -> normalized order-4 (b[0..4], a[0..4]) float64, a[0]=1."""
    sos = np.asarray(sos, dtype=np.float64)
    b1, a1 = sos[0, :3] / sos[0, 3], sos[0, 3:] / sos[0, 3]
    b2, a2 = sos[1, :3] / sos[1, 3], sos[1, 3:] / sos[1, 3]
    return np.convolve(b1, b2), np.convolve(a1, a2)


def _filter_constants(b, a):
    """T_h [L,L], Phi [L,8], M8 [8,8] (float64) for order-4 filter (b, a)."""
    # impulse response h[0..L-1]
    h = np.zeros(L)
    u = np.zeros(L + 4)
    y = np.zeros(L + 4)
    u[4] = 1.0
    for t in range(L):
        acc = b[0] * u[t + 4] + b[1] * u[t + 3] + b[2] * u[t + 2] \
            + b[3] * u[t + 1] + b[4] * u[t]
        acc -= a[1] * y[t + 3] + a[2] * y[t + 2] + a[3] * y[t + 1] + a[4] * y[t]
        y[t + 4] = acc
        h[t] = acc
    T_h = np.zeros((L, L))
    for i in range(L):
        T_h[i:, i] = h[: L - i]
    # Phi[:, j]: zero-input response to unit initial state e_j
    # state layout: j=0..3 -> u[-4+j], j=4..7 -> y[-4+(j-4)]
    Phi = np.zeros((L, 8))
    for j in range(8):
        u = np.zeros(L + 4)
        y = np.zeros(L + 4)
        if j < 4:
            u[j] = 1.0
        else:
            y[j - 4] = 1.0
        for t in range(L):
            acc = b[0] * u[t + 4] + b[1] * u[t + 3] + b[2] * u[t + 2] \
                + b[3] * u[t + 1] + b[4] * u[t]
            acc -= a[1] * y[t + 3] + a[2] * y[t + 2] + a[3] * y[t + 1] + a[4] * y[t]
            y[t + 4] = acc
            Phi[t, j] = acc
    M8 = np.zeros((8, 8))
    M8[4:8, :] = Phi[L - 4 : L, :]
    return T_h, Phi, M8


def _g_diagonals(M8):
    """Gd [NB, 128, 128]: Gd[d][ci*8+q, cj*8+p] = (M8^(d*CB+ci-cj-1))[q,p]."""
    pows = [np.eye(8)]
    for _ in range(NB * CB):
        pows.append(pows[-1] @ M8)
    Gd = np.zeros((NB, 128, 128))
    for d in range(NB):
        for ci in range(CB):
            for cj in range(CB):
                e = d * CB + ci - cj - 1
                if e >= 0:
                    Gd[d, ci * 8 : ci * 8 + 8, cj * 8 : cj * 8 + 8] = pows[e]
    return Gd


def _host_constants(bp_sos, lp_sos, use_g):
    out = {}
    need_g = False
    for name, sos in (("bp", bp_sos), ("lp", lp_sos)):
        bb, aa = _combine_sos(sos)
        T_h, Phi, M8 = _filter_constants(bb, aa)
        # sigma_c == d_{c-1} holds iff the chunk-transition matrix A^L is
        # negligible at fp32 scale; otherwise the exact G path is required.
        if np.abs(M8).max() > 1e-6:
            need_g = True
        out[f"th_{name}"] = np.ascontiguousarray(T_h.T.astype(np.float32))     # lhsT
        out[f"phi_{name}"] = np.ascontiguousarray(Phi.T.astype(np.float32))    # [8,128]
        if use_g:
            Gd = _g_diagonals(M8)
            out[f"gd_{name}"] = np.ascontiguousarray(
                np.concatenate([Gd[d].T for d in range(NB)], axis=1).astype(np.float32)
            )  # [128, NB*128]
    return out, need_g


# ---------------------------------------------------------------- device IR
_PROGRAM_CACHE = {}


def _build_program(mm_dtype_name="float32r", use_g=False):
    import concourse.bass as bass
    import concourse.mybir as mybir
    import concourse.tile as tile
    from concourse import bacc

    F32 = mybir.dt.float32
    MMDT = getattr(mybir.dt, mm_dtype_name)
    ts = bass.ts
    HT = T // 2          # half extent for the tail/sigma buffers
    TLW = HT + S         # tail buffer width per half (128-col overlap)

    nc = bacc.Bacc(None)
    x_d = nc.declare_dram_parameter("x", [128, T], MMDT, isOutput=False)
    th_d = {f: nc.declare_dram_parameter(f"th_{f}", [128, 128], MMDT, False)
            for f in ("bp", "lp")}
    phi_d = {f: nc.declare_dram_parameter(f"phi_{f}", [8, 128], MMDT, False)
             for f in ("bp", "lp")}
    if use_g:
        gd_d = {f: nc.declare_dram_parameter(f"gd_{f}", [128, NB * 128], MMDT, False)
                for f in ("bp", "lp")}
    out_d = nc.declare_dram_parameter("out", [128, T], MMDT, isOutput=True)

    with tile.TileContext(nc) as tc:
        with (
            tc.tile_pool(name="big", bufs=1) as bigpool,
            tc.tile_pool(name="consts", bufs=1) as cpool,
            tc.tile_pool(name="work", bufs=2 if not use_g else 1) as wpool,
            tc.tile_pool(name="tlp", bufs=2) as tlpool,
            tc.tile_pool(name="sig", bufs=1) as sigpool,
            tc.tile_pool(name="psA", bufs=4 if use_g else 8, space=bass.MemorySpace.PSUM) as psA,
            tc.tile_pool(name="psB", bufs=2, space=bass.MemorySpace.PSUM) as psB,
        ):
            bufA = bigpool.tile([128, T], MMDT, tag="bufA", name="bufA")
            bufB = bigpool.tile([128, T], MMDT, tag="bufB", name="bufB")
            th_sb = {f: cpool.tile([128, 128], MMDT, tag=f"th{f}", name=f"th{f}")
                     for f in ("bp", "lp")}
            phi_sb = {f: cpool.tile([8, 128], MMDT, tag=f"phi{f}", name=f"phi{f}")
                      for f in ("bp", "lp")}
            for f in ("bp", "lp"):
                nc.sync.dma_start(out=th_sb[f][:], in_=th_d[f][:])
                nc.sync.dma_start(out=phi_sb[f][:], in_=phi_d[f][:])
            if use_g:
                gd_sb = {f: cpool.tile([128, NB * 128], MMDT, tag=f"gd{f}",
                                       name=f"gd{f}") for f in ("bp", "lp")}
                for f in ("bp", "lp"):
                    nc.sync.dma_start(out=gd_sb[f][:], in_=gd_d[f][:])
            for g in range(16):
                nc.scalar.dma_start(out=bufA[:, ts(g, T // 16)], in_=x_d[:, ts(g, T // 16)])

            def phase_A(IN, YZ, th):
                """Y_zs = T_h @ X, tile by tile, PSUM -> SBUF."""
                for t in range(NT):
                    ps = psA.tile([128, TILE], F32, tag="psA", name="psA_t")
                    nc.tensor.matmul(ps[:], th[:], IN[:, ts(t, TILE)],
                                     start=True, stop=True)
                    if t % 2 == 0:
                        nc.scalar.copy(YZ[:, ts(t, TILE)], ps[:])
                    else:
                        nc.vector.tensor_copy(YZ[:, ts(t, TILE)], ps[:])

            def phase_D(YZ, phi, sig_of_half, square, to_dram):
                """Y += Phi @ sigma in place, then square / store."""
                for h in range(2):
                    sig = sig_of_half(h)
                    for tl in range(NT // 2):
                        t = h * (NT // 2) + tl
                        ps = psA.tile([128, TILE], F32, tag="psA", name="psA_t")
                        nc.tensor.matmul(ps[:], phi[:], sig[:, ts(tl, TILE)],
                                         start=True, stop=True)
                        nc.vector.tensor_add(YZ[:, ts(t, TILE)],
                                             YZ[:, ts(t, TILE)], ps[:])
                        if square:
                            nc.scalar.square(YZ[:, ts(t, TILE)], YZ[:, ts(t, TILE)])
                if to_dram is not None:
                    for g in range(16):
                        nc.sync.dma_start(out=to_dram[:, ts(g, T // 16)],
                                          in_=YZ[:, ts(g, T // 16)])

            def run_filter_fused(IN, OUT, th, phi, square, to_dram):
                """Fused A+D: per tile, psum = T_h@X then += Phi@sigma; the
                zero-state tails (psum rows L-4..L-1) are staged to STRIP and
                gathered (shifted one chunk right) into TL for the next
                group's sigma. sigma_c == d_{c-1} since |A^L| ~ 0.

                Software-pipelined in groups of GT tiles: group g's correction
                matmuls run after group g's strip/TL are built; group g+1's
                T_h matmuls fill the other half of PSUM meanwhile.
                """
                GT = 4                      # tiles per group
                GW = GT * TILE              # 2048 cols per group
                NG = NT // GT               # 16 groups
                state = {}                  # per-group: (pss, tl)

                def sdma(out_ap, in_ap, k=16):
                    # Split the free dim so descriptors round-robin across
                    # all 16 SDMA engines instead of pinning to engines 0-3.
                    w = out_ap.shape[-1]
                    if w % k == 0 and len(out_ap.shape) == 2 and len(in_ap.shape) == 2:
                        out_ap = out_ap.rearrange("p (k w) -> p k w", k=k)
                        in_ap = in_ap.rearrange("p (k w) -> p k w", k=k)
                    nc.sync.dma_start(out=out_ap, in_=in_ap)

                def emit_mm1(g):
                    strip = wpool.tile([128, GW], MMDT, tag="strip", name="strip")
                    pss = []
                    for j in range(GT):
                        t = g * GT + j
                        ps = psA.tile([128, TILE], F32, tag="psA", name="psA_t")
                        nc.tensor.matmul(ps[:], th[:], IN[:, ts(t, TILE)],
                                         start=True, stop=False,
                                         skip_group_check=True)
                        if (j % 2 == 0) == square:
                            nc.vector.tensor_copy(strip[96:128, ts(j, TILE)],
                                                  ps[96:128, :])
                        else:
                            nc.scalar.copy(strip[96:128, ts(j, TILE)], ps[96:128, :])
                        pss.append(ps)
                    tl = tlpool.tile([8, GW], MMDT, tag="tl", name="tl")
                    if g == 0:
                        nc.vector.memzero(tl[:, 0:S])
                        sdma(tl[0:4, S:GW], IN[L - 4 : L, 0 : GW - S])
                        sdma(tl[4:8, S:GW], strip[L - 4 : L, 0 : GW - S])
                    else:
                        prev_strip = state[g - 1][2]
                        sdma(tl[0:4, :],
                             IN[L - 4 : L, g * GW - S : (g + 1) * GW - S])
                        nc.sync.dma_start(out=tl[4:8, 0:S],
                                          in_=prev_strip[L - 4 : L, GW - S : GW])
                        sdma(tl[4:8, S:GW], strip[L - 4 : L, 0 : GW - S])
                    state[g] = (pss, tl, strip)

                def emit_mm2(g):
                    pss, tl, _ = state[g]
                    for j in range(GT):
                        t = g * GT + j
                        ps = pss[j]
                        nc.tensor.matmul(ps[:], phi[:], tl[:, ts(j, TILE)],
                                         start=False, stop=True,
                                         skip_group_check=True)
                        if square:
                            nc.scalar.square(OUT[:, ts(t, TILE)], ps[:])
                        else:
                            if j % 2 == 0:
                                nc.vector.tensor_copy(OUT[:, ts(t, TILE)], ps[:])
                            else:
                                nc.scalar.copy(OUT[:, ts(t, TILE)], ps[:])
                    del state[g]

                emit_mm1(0)
                for g in range(1, NG):
                    emit_mm1(g)
                    emit_mm2(g - 1)
                emit_mm2(NG - 1)
                if to_dram is not None:
                    for g in range(16):
                        nc.scalar.dma_start(out=to_dram[:, ts(g, T // 16)],
                                            in_=OUT[:, ts(g, T // 16)])

            def run_filter_g(IN, YZ, th, phi, gd, square, to_dram):
                """Exact path: states via block-Toeplitz diagonal matmuls."""
                phase_A(IN, YZ, th)
                # Dm [ci*8+q, (cb, s)]
                dm = wpool.tile([128, NB * S], MMDT, tag="dm", name="dm")
                in4 = IN[:].rearrange("p (cb ci s) -> p cb ci s", cb=NB, ci=CB, s=S)
                yz4 = YZ[:].rearrange("p (cb ci s) -> p cb ci s", cb=NB, ci=CB, s=S)
                dm3 = dm[:].rearrange("p (cb s) -> p cb s", cb=NB, s=S)
                for ci in range(CB):
                    nc.sync.dma_start(out=dm3[ci * 8 : ci * 8 + 4],
                                      in_=in4[L - 4 : L, :, ci, :])
                    nc.sync.dma_start(out=dm3[ci * 8 + 4 : ci * 8 + 8],
                                      in_=yz4[L - 4 : L, :, ci, :])
                sp = psB.tile([128, 2 * TILE], F32, tag="spsum", name="spsum")
                for d in range(4):
                    nc.tensor.matmul(sp[:, d * S : 4 * S], gd[:, ts(d, 128)],
                                     dm[:, 0 : (4 - d) * S],
                                     start=(d == 0), stop=(d == 3))
                for d in range(8):
                    lo = max(4, d)
                    nc.tensor.matmul(
                        sp[:, TILE + (lo - 4) * S : TILE + 4 * S],
                        gd[:, ts(d, 128)], dm[:, (lo - d) * S : (8 - d) * S],
                        start=(d == 0), stop=(d == 7))
                sblk = wpool.tile([128, NB * S], MMDT, tag="sblk", name="sblk")
                nc.vector.tensor_copy(sblk[:], sp[:])

                def sig_of_half(h):
                    sig = sigpool.tile([8, HT], MMDT, tag="tl", name="sig",
                                       padded_shape=[8, TLW])
                    sig3 = sig[:].rearrange("p (cb ci s) -> p cb ci s",
                                            cb=NB // 2, ci=CB, s=S)
                    for q in range(8):
                        for cbl in range(NB // 2):
                            cb = h * (NB // 2) + cbl
                            nc.sync.dma_start(
                                out=sig3[q : q + 1, cbl],
                                in_=sblk[q::8, cb * S : (cb + 1) * S])
                    return sig

                phase_D(YZ, phi, sig_of_half, square, to_dram)

            if use_g:
                run_filter_g(bufA, bufB, th_sb["bp"], phi_sb["bp"], gd_sb["bp"],
                             square=True, to_dram=None)
                run_filter_g(bufB, bufA, th_sb["lp"], phi_sb["lp"], gd_sb["lp"],
                             square=False, to_dram=out_d)
            else:
                run_filter_fused(bufA, bufB, th_sb["bp"], phi_sb["bp"],
                                 square=True, to_dram=None)
                run_filter_fused(bufB, bufA, th_sb["lp"], phi_sb["lp"],
                                 square=False, to_dram=out_d)

    nc.compile()
    return nc


def _get_program(mm_dtype_name="float32r", use_g=False):
    key = (mm_dtype_name, use_g)
    if key not in _PROGRAM_CACHE:
        _PROGRAM_CACHE[key] = _build_program(mm_dtype_name, use_g)
    return _PROGRAM_CACHE[key]


# ---------------------------------------------------------------- host entry
def _shard_inputs(x):
    """x [B,T,C] -> list of per-core Xm [128, T] fp32 arrays."""
    xs = np.ascontiguousarray(np.transpose(np.asarray(x, dtype=np.float32),
                                           (0, 2, 1))).reshape(B * C, T)
    shards = []
    for core in range(NCORES):
        seqs = xs[core * S : (core + 1) * S]
        Xm = np.ascontiguousarray(
            seqs.reshape(S, N, L).transpose(2, 1, 0)).reshape(L, N * S)
        shards.append(Xm)
    return shards


def _unshard_output(outs):
    """list of per-core [128, T] device outputs -> [B, T, C] fp32."""
    ys = np.empty((B * C, T), dtype=np.float32)
    for core in range(NCORES):
        O = np.asarray(outs[core])
        ys[core * S : (core + 1) * S] = (
            O.reshape(L, N, S).transpose(2, 1, 0).reshape(S, T))
    return np.ascontiguousarray(ys.reshape(B, C, T).transpose(0, 2, 1))


def kernel(x, bp_sos, lp_sos, _trace=False, _mm_dtype="float32r", _force_g=False):
    from concourse.bass_utils import run_bass_kernel_spmd

    consts, need_g = _host_constants(np.asarray(bp_sos), np.asarray(lp_sos),
                                     use_g=_force_g)
    if need_g and not _force_g:
        return kernel(x, bp_sos, lp_sos, _trace=_trace, _mm_dtype=_mm_dtype,
                      _force_g=True)
    shards = _shard_inputs(x)
    nc = _get_program(_mm_dtype, use_g=_force_g)
    in_maps = [dict(consts, x=shards[core]) for core in range(NCORES)]
    res = run_bass_kernel_spmd(nc, in_maps, list(range(NCORES)), trace=_trace)
    out = _unshard_output([res.results[core]["out"] for core in range(NCORES)])
    if _trace:
        return out, res
    return out


if __name__ == "__main__":
    rng = np.random.default_rng(0)
    x = rng.standard_normal((B, T, C), dtype=np.float32)
    print("smoke: shard/unshard roundtrip")
    sh = _shard_inputs(x)
    rt = _unshard_output([
        np.ascontiguousarray(s.reshape(L, N, S).transpose(0, 1, 2)) for s in sh])
    # identity check of layout plumbing: unshard(shard-layout passthrough)
    # maps Xm back through the output path; compare against x directly.
    print("roundtrip ok:", np.array_equal(rt, x))

